# revision 9
# baseline (speedup 1.0000x reference)
# Trainium2 Bass kernel for the CPC 'same'-mode InfoNCE loss (nn_CPC_22514218566439).
#
# Math (per inner step s and prediction offset k):
#   H   = enc[T0+k+s] @ Wk[k]          [B, L]   (Wk stored [m, l]; pred = ctx @ Wk.T)
#   sim = H @ ctx[T_IN+s].T            [B, B]   sim[b, c] = <enc_b @ Wk, ctx_c>
#   logp = log_softmax(sim, axis=-1)
#   loss += sum_b logp[b, b];  correct += #{c : argmax_b logp[b, c] == c}
#
# Sharding: data-parallel over the 103 inner steps across 8 NeuronCores
# (13 steps/core; core 7 computes one padded step the host discards).
# Each core stages per-(pair, b-half) columns of: -rowmax, sumexp (softmax
# stats for the host-side lse), the sim diagonal, and the argmax-correct
# indicator.  The host sums valid columns; no on-device collectives.
#
# Per (s, k) pair on device (native ops only — no custom DVE ucode):
#   PE : 16 matmuls HT[l,b] += Wk_chunk x encT   (contract m, lhsT = Wk as stored)
#        8 matmuls  sim[b,c] += HT_chunk x ctxT  (contract l)
#        4 matmuls  PT = P_block^T x I           (transpose for column max)
#   ACT: HT psum->sbuf copies, exp(sim - rowmax) with fused row-sum,
#        exp of the extracted diagonal
#   DVE: -rowmax reduce, reciprocal, column max, diagonal mult+sum, compares
#   GPS: P = expo * recip' (row-normalize, tilted)
#
# argmax tie-breaking: jnp.argmax picks the FIRST max index.  Softmax rows
# with a dominant max produce exact 0.0 log-softmax ties (log1p rounds to 0),
# so ties are common.  We compare in probability space with a per-row tilt
# recip'[b] = recip[b] * (1 + (255-b)*2^-22): exact ties then resolve toward
# the earlier row, matching first-index semantics; the tilt is far below the
# scale of any genuine gap.  The diagonal probability d' is recomputed from
# the extracted sim diagonal through the identical ACT-exp + fp32-multiply
# path, so d' == P'[c,c] bit-exactly and `d' >= colmax(P')` is an exact
# attains-the-max test.

import os
import numpy as np

S, B, L, K = 128, 256, 512, 8
T_IN = 16
STEPS = S - T_IN - (K + 1)      # 103
T0 = T_IN + 1                   # 17
NCORES = 8
SPC = 13                        # steps per core (8*13 = 104 >= 103)
NT = SPC + K - 1                # 20 enc time slices each core needs
F32 = np.float32

_CACHE = {}


def _build_nc(spc):
    from contextlib import ExitStack
    import concourse.bacc as bacc
    import concourse.tile as tile
    from concourse import mybir
    from concourse.masks import make_identity

    f32 = mybir.dt.float32
    i32 = mybir.dt.int32
    AF = mybir.ActivationFunctionType
    OP = mybir.AluOpType
    AX = mybir.AxisListType.X

    nt = spc + K - 1
    npair = spc * K
    ncols = 2 * npair

    nc = bacc.Bacc("TRN2")
    enc_d = nc.declare_dram_parameter("enc", [nt, B, L], f32, isOutput=False)
    ctx_d = nc.declare_dram_parameter("ctx", [spc, B, L], f32, isOutput=False)
    wk_d = nc.declare_dram_parameter("wk", [K, L, L], f32, isOutput=False)
    nmax_d = nc.declare_dram_parameter("negmax", [128, ncols], f32, isOutput=True)
    sexp_d = nc.declare_dram_parameter("sumexp", [128, ncols], f32, isOutput=True)
    diag_d = nc.declare_dram_parameter("simdiag", [128, ncols], f32, isOutput=True)
    corr_d = nc.declare_dram_parameter("corr", [128, ncols], f32, isOutput=True)

    with tile.TileContext(nc) as tc, ExitStack() as ctx:
        const = ctx.enter_context(tc.tile_pool(name="const", bufs=1))
        stage = ctx.enter_context(tc.tile_pool(name="stage", bufs=1))
        wkp = ctx.enter_context(tc.tile_pool(name="wkp", bufs=1))
        encT_p = ctx.enter_context(tc.tile_pool(name="encT", bufs=K + 2))
        ctxT_p = ctx.enter_context(tc.tile_pool(name="ctxT", bufs=2))
        raw_p = ctx.enter_context(tc.tile_pool(name="raw", bufs=3))
        htsb_p = ctx.enter_context(tc.tile_pool(name="htsb", bufs=4))
        expo_p = ctx.enter_context(tc.tile_pool(name="expo", bufs=3))
        p_p = ctx.enter_context(tc.tile_pool(name="pp", bufs=3))
        junk_p = ctx.enter_context(tc.tile_pool(name="junk", bufs=3))
        small_p = ctx.enter_context(tc.tile_pool(name="small", bufs=6))
        ht_ps = ctx.enter_context(tc.tile_pool(name="htps", bufs=4, space="PSUM"))
        sim_ps = ctx.enter_context(tc.tile_pool(name="simps", bufs=2, space="PSUM"))
        at_ps = ctx.enter_context(tc.tile_pool(name="atps", bufs=1, space="PSUM"))
        tr_ps = ctx.enter_context(tc.tile_pool(name="trps", bufs=1, space="PSUM"))

        # ---- constants -------------------------------------------------
        ident = const.tile([128, 128], f32)
        make_identity(nc, ident)
        # diagmask[p, col] = 1.0 at the diagonal position of each b-half:
        # g=0 -> col p (within cols 0:256), g=1 -> col 384+p (within 256:512)
        diagmask = const.tile([128, 512], f32)
        nc.gpsimd.memset(diagmask, 0.0)
        for g, off in ((0, 0), (1, 384)):
            nc.gpsimd.affine_select(
                out=diagmask[:, off:off + 128],
                in_=diagmask[:, off:off + 128],
                compare_op=OP.not_equal,
                fill=1.0,
                base=0,
                pattern=[[-1, 128]],
                channel_multiplier=1,
            )
        # per-row tie tilt: w[b] = 1 + (255 - b) * 2^-22, b = g*128 + p
        bidx_i = const.tile([128, 2], i32)
        nc.gpsimd.iota(bidx_i, pattern=[[128, 2]], base=0, channel_multiplier=1)
        bidx_f = const.tile([128, 2], f32)
        nc.vector.tensor_copy(out=bidx_f, in_=bidx_i)
        wtilt = const.tile([128, 2], f32)
        nc.vector.tensor_scalar(
            out=wtilt, in0=bidx_f,
            scalar1=-(2.0 ** -22), scalar2=1.0 + 255.0 * 2.0 ** -22,
            op0=OP.mult, op1=OP.add,
        )

        negmax_sb = stage.tile([128, ncols], f32)
        sumexp_sb = stage.tile([128, ncols], f32)
        simdiag_sb = stage.tile([128, ncols], f32)
        corr_sb = stage.tile([128, ncols], f32)

        # ---- weights: Wk[k] as [m(part), l(free)] chunks ---------------
        wk_sb = wkp.tile([128, K, 4, L], f32)
        for k in range(K):
            for mc in range(4):
                nc.sync.dma_start(
                    out=wk_sb[:, k, mc, :],
                    in_=wk_d[k, mc * 128:(mc + 1) * 128, :],
                )

        # ---- transposed load of a [256, 512] DRAM matrix ---------------
        # result tile [128, 4, 256]: chunk c holds rows 128c..128c+127 of
        # the transposed matrix (free dim = original row index 0..255).
        def load_T(dram_mat, pool):
            out_t = pool.tile([128, 4, 256], f32)
            raw = raw_p.tile([128, 2, 512], f32)
            nc.sync.dma_start(out=raw, in_=dram_mat.rearrange("(h p) m -> p h m", p=128))
            for half in range(2):
                tp = tr_ps.tile([128, 512], f32)
                for j in range(2):
                    mc = half * 2 + j
                    for h in range(2):
                        nc.tensor.matmul(
                            tp[:, j * 256 + h * 128: j * 256 + (h + 1) * 128],
                            lhsT=raw[:, h, mc * 128:(mc + 1) * 128],
                            rhs=ident,
                            start=True, stop=True,
                        )
                nc.scalar.copy(
                    out=out_t[:, half * 2:(half + 1) * 2, :],
                    in_=tp.rearrange("p (a b) -> p a b", a=2),
                )
            return out_t

        # ---- one (s, k) pair -------------------------------------------
        def pair(s, k, ctxT, encT_t):
            pcol = 2 * (s * K + k)

            # HT[l, b] = sum_m Wk[k][m, l] * enc[t][b, m]   (4 l-chunks)
            ht_a = ht_ps.tile([128, 512], f32, tag="ht")
            ht_b = ht_ps.tile([128, 512], f32, tag="ht")
            for lt in range(4):
                dst = (ht_a if lt < 2 else ht_b)[:, (lt % 2) * 256:(lt % 2) * 256 + 256]
                for mc in range(4):
                    nc.tensor.matmul(
                        dst,
                        lhsT=wk_sb[:, k, mc, lt * 128:(lt + 1) * 128],
                        rhs=encT_t[:, mc, :],
                        start=(mc == 0), stop=(mc == 3),
                    )
            ht_sb = htsb_p.tile([128, 2, 512], f32)
            nc.scalar.copy(out=ht_sb[:, 0, :], in_=ht_a)
            nc.scalar.copy(out=ht_sb[:, 1, :], in_=ht_b)

            # sim[b, c] = sum_l HT[l, b] * ctxT[l, c]   (2 b-halves g)
            sim = sim_ps.tile([128, 512], f32)
            for g in range(2):
                dst = sim[:, g * 256:(g + 1) * 256]
                for lc in range(4):
                    nc.tensor.matmul(
                        dst,
                        lhsT=ht_sb[:, lc // 2, (lc % 2) * 256 + g * 128:
                                   (lc % 2) * 256 + g * 128 + 128],
                        rhs=ctxT[:, lc, :],
                        start=(lc == 0), stop=(lc == 3),
                    )

            # -rowmax -> staging (negate folds the exp-bias negation)
            nc.vector.reduce_max(
                out=negmax_sb[:, pcol:pcol + 2],
                in_=sim.rearrange("p (g c) -> p g c", g=2),
                axis=AX,
                negate=True,
            )
            # expo = exp(sim - rowmax), fused row-sums -> staging
            expo = expo_p.tile([128, 512], f32)
            for g in range(2):
                nc.scalar.activation(
                    out=expo[:, g * 256:(g + 1) * 256],
                    in_=sim[:, g * 256:(g + 1) * 256],
                    func=AF.Exp,
                    bias=negmax_sb[:, pcol + g:pcol + g + 1],
                    scale=1.0,
                    accum_out=sumexp_sb[:, pcol + g:pcol + g + 1],
                )
            # tilted row reciprocal
            rec = small_p.tile([128, 2], f32, tag="rec")
            nc.vector.reciprocal(out=rec, in_=sumexp_sb[:, pcol:pcol + 2])
            recw = small_p.tile([128, 2], f32, tag="recw")
            nc.vector.tensor_mul(out=recw, in0=rec, in1=wtilt)
            # P' = expo * recip'   (softmax probabilities, tilted)
            pp = p_p.tile([128, 512], f32)
            for g in range(2):
                nc.gpsimd.tensor_scalar_mul(
                    out=pp[:, g * 256:(g + 1) * 256],
                    in0=expo[:, g * 256:(g + 1) * 256],
                    scalar1=recw[:, g:g + 1],
                )
            # PT' = transpose(P') via matmul against identity
            pt = at_ps.tile([128, 512], f32)
            for h in range(2):
                for g in range(2):
                    nc.tensor.matmul(
                        pt[:, h * 256 + g * 128: h * 256 + (g + 1) * 128],
                        lhsT=pp[:, g * 256 + h * 128: g * 256 + h * 128 + 128],
                        rhs=ident,
                        start=True, stop=True,
                    )
            cmx = small_p.tile([128, 2], f32, tag="cmx")
            nc.vector.reduce_max(
                out=cmx,
                in_=pt.rearrange("p (g c) -> p g c", g=2),
                axis=AX,
            )
            # sim diagonal -> staging (exact: mult by {0,1} mask, sum zeros)
            junk = junk_p.tile([128, 512], f32, tag="junk")
            nc.vector.tensor_tensor(out=junk, in0=sim[:, :], in1=diagmask, op=OP.mult)
            nc.vector.reduce_sum(
                out=simdiag_sb[:, pcol:pcol + 2],
                in_=junk.rearrange("p (g c) -> p g c", g=2),
                axis=AX,
            )
            # d' = exp(simdiag - rowmax) * recip'  (bit-identical to P'[c,c])
            ed = small_p.tile([128, 2], f32, tag="ed")
            for g in range(2):
                nc.scalar.activation(
                    out=ed[:, g:g + 1],
                    in_=simdiag_sb[:, pcol + g:pcol + g + 1],
                    func=AF.Exp,
                    bias=negmax_sb[:, pcol + g:pcol + g + 1],
                    scale=1.0,
                )
            dp = small_p.tile([128, 2], f32, tag="dp")
            nc.vector.tensor_mul(out=dp, in0=ed, in1=recw)
            nc.vector.tensor_tensor(
                out=corr_sb[:, pcol:pcol + 2], in0=dp, in1=cmx, op=OP.is_ge)

        # ---- main loop --------------------------------------------------
        encT = {}
        for t in range(K):
            encT[t] = load_T(enc_d[t], encT_p)
        for s in range(spc):
            ctxT = load_T(ctx_d[s], ctxT_p)
            tmax = s + K - 1
            if tmax not in encT:
                encT[tmax] = load_T(enc_d[tmax], encT_p)
            for k in range(K):
                pair(s, k, ctxT, encT[s + k])
            if s - 1 in encT:
                del encT[s - 1]

        nc.sync.dma_start(out=nmax_d[:, :], in_=negmax_sb)
        nc.sync.dma_start(out=sexp_d[:, :], in_=sumexp_sb)
        nc.sync.dma_start(out=diag_d[:, :], in_=simdiag_sb)
        nc.sync.dma_start(out=corr_d[:, :], in_=corr_sb)

    nc.compile()
    return nc


def _get_nc(spc=SPC):
    if spc not in _CACHE:
        _CACHE[spc] = _build_nc(spc)
    return _CACHE[spc]


LAST_RESULTS = None  # test harness can inspect exec_time_ns / profile


def _install_ntff_hook_shim():
    """Register the NTFF profiling hook (antenv.axon_hooks shim) so
    run_bass_kernel_spmd(trace=True) can capture a profile under axon.
    Dev-only; the graded path never calls this."""
    import sys
    import types
    import ctypes
    import contextlib

    if "antenv.axon_hooks" in sys.modules:
        return
    so_path = "/opt/axon/libaxon_pjrt.so"
    try:
        lib = ctypes.CDLL(so_path)
    except OSError:
        return
    if not hasattr(lib, "axon_start_nrt_profile"):
        return
    lib.axon_start_nrt_profile.argtypes = [ctypes.POINTER(ctypes.c_int64), ctypes.c_size_t]
    lib.axon_start_nrt_profile.restype = ctypes.c_int64
    lib.axon_stop_nrt_profile.argtypes = [ctypes.c_char_p]
    lib.axon_stop_nrt_profile.restype = ctypes.c_int64

    @contextlib.contextmanager
    def _hook(output_dir, device_ids):
        import jax
        jax.devices()
        if device_ids:
            ids = (ctypes.c_int64 * len(device_ids))(*device_ids)
            rc = lib.axon_start_nrt_profile(ids, len(device_ids))
        else:
            rc = lib.axon_start_nrt_profile(None, 0)
        if rc != 0:
            raise RuntimeError(f"axon_start_nrt_profile rc={rc}")
        try:
            yield
        finally:
            n = lib.axon_stop_nrt_profile(str(output_dir).encode())
            print(f"ntff profile: {n} file(s) written to {output_dir}")

    holder = [_hook]
    mod = types.ModuleType("antenv.axon_hooks")
    mod.get_axon_ntff_profile_hook = lambda: holder[0]
    mod.set_axon_ntff_profile_hook = lambda h: holder.__setitem__(0, h)
    sys.modules["antenv.axon_hooks"] = mod


def kernel(**inputs):
    global LAST_RESULTS
    enc = np.ascontiguousarray(np.asarray(inputs["encoded_x"], dtype=F32))
    ctxf = np.ascontiguousarray(np.asarray(inputs["context"], dtype=F32))
    wk = np.ascontiguousarray(np.asarray(inputs["Wk"], dtype=F32))
    t_in = int(inputs["timesteps_in"])
    k_out = int(inputs["timesteps_out"])
    t_ign = int(inputs["timesteps_ignore"])
    assert enc.shape == (S, B, L) and ctxf.shape == (S, B, L)
    assert wk.shape == (K, L, L)
    assert (t_in, k_out, t_ign) == (T_IN, K, 0), "kernel hardcodes these"

    from concourse.bass_utils import run_bass_kernel_spmd

    trace = bool(int(os.environ.get("CPC_TRACE", "0")))
    if trace:
        _install_ntff_hook_shim()

    nc = _get_nc()

    in_maps = []
    for i in range(NCORES):
        s0 = SPC * i
        # core 7's slices stay in range: T0 + 91 + 20 == 128
        in_maps.append({
            "enc": enc[T0 + s0: T0 + s0 + NT],
            "ctx": ctxf[T_IN + s0: T_IN + s0 + SPC],
            "wk": wk,
        })

    res = run_bass_kernel_spmd(nc, in_maps, list(range(NCORES)), trace=trace)
    LAST_RESULTS = res

    denom = B * K * STEPS
    diag_total = 0.0
    lse_total = 0.0
    corr_total = 0.0
    for i in range(NCORES):
        nvalid = 2 * K * min(SPC, STEPS - SPC * i)
        r = res.results[i]
        diag_total += r["simdiag"][:, :nvalid].astype(np.float64).sum()
        nmax = r["negmax"][:, :nvalid].astype(np.float64)
        sexp = r["sumexp"][:, :nvalid].astype(np.float64)
        lse_total += (-nmax + np.log(sexp)).sum()
        corr_total += r["corr"][:, :nvalid].astype(np.float64).sum()

    loss = np.float32(-(diag_total - lse_total) / denom)
    accuracy = np.float32(corr_total / denom)
    return (accuracy, loss)


# revision 21
# speedup vs baseline: 4.7086x; 4.7086x over previous
# Trainium2 Bass kernel for the CPC 'same'-mode InfoNCE loss (nn_CPC_22514218566439).
#
# Math (per inner step s and prediction offset k):
#   H   = enc[T0+k+s] @ Wk[k]          [B, L]   (Wk stored [m, l]; pred = ctx @ Wk.T)
#   sim = H @ ctx[T_IN+s].T            [B, B]   sim[b, c] = <enc_b @ Wk, ctx_c>
#   logp = log_softmax(sim, axis=-1)
#   loss += sum_b logp[b, b];  correct += #{c : argmax_b logp[b, c] == c}
#
# Sharding: data-parallel over the 103 inner steps across 8 NeuronCores
# (13 steps/core; core 7 computes one padded step the host discards).
# Each core stages per-(pair, b-half) columns of: -rowmax, sumexp (softmax
# stats for the host-side lse), the sim diagonal, and the argmax-correct
# indicator.  The host sums valid columns; no on-device collectives.
#
# Per (s, k) pair on device (native ops only — no custom DVE ucode):
#   PE : 16 matmuls HT[l,b] += Wk_chunk x encT   (contract m, lhsT = Wk as stored)
#        8 matmuls  sim[b,c] += HT_chunk x ctxT  (contract l)
#        4 matmuls  PT = P_block^T x I           (transpose for column max)
#   ACT: HT psum->sbuf copies, exp(sim - rowmax) with fused row-sum,
#        exp of the extracted diagonal
#   DVE: -rowmax reduce, reciprocal, column max, diagonal mult+sum, compares
#   GPS: P = expo * recip' (row-normalize, tilted)
#
# argmax tie-breaking: jnp.argmax picks the FIRST max index.  Softmax rows
# with a dominant max produce exact 0.0 log-softmax ties (log1p rounds to 0),
# so ties are common.  We compare in probability space with a per-row tilt
# recip'[b] = recip[b] * (1 + (255-b)*2^-22): exact ties then resolve toward
# the earlier row, matching first-index semantics; the tilt is far below the
# scale of any genuine gap.  The diagonal probability d' is recomputed from
# the extracted sim diagonal through the identical ACT-exp + fp32-multiply
# path, so d' == P'[c,c] bit-exactly and `d' >= colmax(P')` is an exact
# attains-the-max test.

import os
import numpy as np

S, B, L, K = 128, 256, 512, 8
T_IN = 16
STEPS = S - T_IN - (K + 1)      # 103
T0 = T_IN + 1                   # 17
NCORES = 8
SPC = 13                        # steps per core (8*13 = 104 >= 103)
NT = SPC + K - 1                # 20 enc time slices each core needs
F32 = np.float32

_CACHE = {}


def _build_nc(spc):
    from contextlib import ExitStack
    import concourse.bacc as bacc
    import concourse.tile as tile
    from concourse import mybir
    from concourse.masks import make_identity

    f32 = mybir.dt.float32
    f32r = mybir.dt.float32r   # TF32-class matmul: 1 cyc/row at N>=256 vs fp32's 4
    i32 = mybir.dt.int32
    AF = mybir.ActivationFunctionType
    OP = mybir.AluOpType
    AX = mybir.AxisListType.X

    nt = spc + K - 1
    npair = spc * K
    ncols = 2 * npair

    nc = bacc.Bacc("TRN2")
    enc_d = nc.declare_dram_parameter("enc", [nt, B, L], f32, isOutput=False)
    ctx_d = nc.declare_dram_parameter("ctx", [spc, B, L], f32, isOutput=False)
    wk_d = nc.declare_dram_parameter("wk", [K, L, L], f32, isOutput=False)
    nmax_d = nc.declare_dram_parameter("negmax", [128, ncols], f32, isOutput=True)
    sexp_d = nc.declare_dram_parameter("sumexp", [128, ncols], f32, isOutput=True)
    diag_d = nc.declare_dram_parameter("simdiag", [128, ncols], f32, isOutput=True)
    corr_d = nc.declare_dram_parameter("corr", [128, ncols], f32, isOutput=True)

    with tile.TileContext(nc) as tc, ExitStack() as ctx:
        const = ctx.enter_context(tc.tile_pool(name="const", bufs=1))
        stage = ctx.enter_context(tc.tile_pool(name="stage", bufs=1))
        wkp = ctx.enter_context(tc.tile_pool(name="wkp", bufs=1))
        encT_p = ctx.enter_context(tc.tile_pool(name="encT", bufs=K + 2))
        ctxT_p = ctx.enter_context(tc.tile_pool(name="ctxT", bufs=2))
        raw_p = ctx.enter_context(tc.tile_pool(name="raw", bufs=3))
        htsb_p = ctx.enter_context(tc.tile_pool(name="htsb", bufs=4))
        expo_p = ctx.enter_context(tc.tile_pool(name="expo", bufs=3))
        p_p = ctx.enter_context(tc.tile_pool(name="pp", bufs=3))
        junk_p = ctx.enter_context(tc.tile_pool(name="junk", bufs=3))
        small_p = ctx.enter_context(tc.tile_pool(name="small", bufs=6))
        ht_ps = ctx.enter_context(tc.tile_pool(name="htps", bufs=4, space="PSUM"))
        sim_ps = ctx.enter_context(tc.tile_pool(name="simps", bufs=2, space="PSUM"))
        at_ps = ctx.enter_context(tc.tile_pool(name="atps", bufs=1, space="PSUM"))
        tr_ps = ctx.enter_context(tc.tile_pool(name="trps", bufs=1, space="PSUM"))

        # ---- constants -------------------------------------------------
        ident = const.tile([128, 128], f32)
        make_identity(nc, ident)
        # diagmask[p, col] = 1.0 at the diagonal position of each b-half:
        # g=0 -> col p (within cols 0:256), g=1 -> col 384+p (within 256:512)
        diagmask = const.tile([128, 512], f32)
        nc.gpsimd.memset(diagmask, 0.0)
        for g, off in ((0, 0), (1, 384)):
            nc.gpsimd.affine_select(
                out=diagmask[:, off:off + 128],
                in_=diagmask[:, off:off + 128],
                compare_op=OP.not_equal,
                fill=1.0,
                base=0,
                pattern=[[-1, 128]],
                channel_multiplier=1,
            )
        # per-row tie tilt: w[b] = 1 + (255 - b) * 2^-22, b = g*128 + p
        bidx_i = const.tile([128, 2], i32)
        nc.gpsimd.iota(bidx_i, pattern=[[128, 2]], base=0, channel_multiplier=1)
        bidx_f = const.tile([128, 2], f32)
        nc.vector.tensor_copy(out=bidx_f, in_=bidx_i)
        wtilt = const.tile([128, 2], f32)
        nc.vector.tensor_scalar(
            out=wtilt, in0=bidx_f,
            scalar1=-(2.0 ** -22), scalar2=1.0 + 255.0 * 2.0 ** -22,
            op0=OP.mult, op1=OP.add,
        )

        negmax_sb = stage.tile([128, ncols], f32)
        sumexp_sb = stage.tile([128, ncols], f32)
        simdiag_sb = stage.tile([128, ncols], f32)
        corr_sb = stage.tile([128, ncols], f32)

        # ---- weights: Wk[k] as [m(part), l(free)] chunks, rounded to f32r
        wk_sb = wkp.tile([128, K, 4, L], f32r)
        for k in range(K):
            for mc in range(4):
                wst = raw_p.tile([128, L], f32, tag="wstage")
                nc.sync.dma_start(
                    out=wst,
                    in_=wk_d[k, mc * 128:(mc + 1) * 128, :],
                )
                nc.vector.tensor_copy(out=wk_sb[:, k, mc, :], in_=wst)

        # ---- transposed load of a [256, 512] DRAM matrix ---------------
        # result tile [128, 4, 256]: chunk c holds rows 128c..128c+127 of
        # the transposed matrix (free dim = original row index 0..255).
        # PE transpose-mode (2 cyc/row fp32); PSUM->SBUF moves go to DMA.
        def load_T(dram_mat, pool):
            out_t = pool.tile([128, 4, 256], f32r)
            raw = raw_p.tile([128, 2, 512], f32)
            nc.sync.dma_start(out=raw, in_=dram_mat.rearrange("(h p) m -> p h m", p=128))
            for half in range(2):
                tp = tr_ps.tile([128, 512], f32)
                for j in range(2):
                    mc = half * 2 + j
                    for h in range(2):
                        nc.tensor.transpose(
                            tp[:, j * 256 + h * 128: j * 256 + (h + 1) * 128],
                            in_=raw[:, h, mc * 128:(mc + 1) * 128],
                            identity=ident,
                        )
                nc.scalar.copy(
                    out=out_t[:, half * 2:(half + 1) * 2, :],
                    in_=tp.rearrange("p (a b) -> p a b", a=2),
                )
            return out_t

        # ---- one (s, k) pair -------------------------------------------
        def pair(s, k, ctxT, encT_t):
            pcol = 2 * (s * K + k)

            # HT[l, b] = sum_m Wk[k][m, l] * enc[t][b, m]   (4 l-chunks)
            # f32r operands: TF32-class precision, 4x the fp32 matmul rate.
            ht_a = ht_ps.tile([128, 512], f32, tag="ht")
            ht_b = ht_ps.tile([128, 512], f32, tag="ht")
            for lt in range(4):
                dst = (ht_a if lt < 2 else ht_b)[:, (lt % 2) * 256:(lt % 2) * 256 + 256]
                for mc in range(4):
                    nc.tensor.matmul(
                        dst,
                        lhsT=wk_sb[:, k, mc, lt * 128:(lt + 1) * 128],
                        rhs=encT_t[:, mc, :],
                        start=(mc == 0), stop=(mc == 3),
                    )
            ht_sb = htsb_p.tile([128, 2, 512], f32r)
            nc.scalar.copy(out=ht_sb[:, 0, :], in_=ht_a)
            nc.scalar.copy(out=ht_sb[:, 1, :], in_=ht_b)

            # sim[b, c] = sum_l HT[l, b] * ctxT[l, c]   (2 b-halves g)
            sim = sim_ps.tile([128, 512], f32)
            for g in range(2):
                dst = sim[:, g * 256:(g + 1) * 256]
                for lc in range(4):
                    nc.tensor.matmul(
                        dst,
                        lhsT=ht_sb[:, lc // 2, (lc % 2) * 256 + g * 128:
                                   (lc % 2) * 256 + g * 128 + 128],
                        rhs=ctxT[:, lc, :],
                        start=(lc == 0), stop=(lc == 3),
                    )

            # -rowmax -> staging (negate folds the exp-bias negation)
            nc.vector.reduce_max(
                out=negmax_sb[:, pcol:pcol + 2],
                in_=sim.rearrange("p (g c) -> p g c", g=2),
                axis=AX,
                negate=True,
            )
            # expo = exp(sim - rowmax), fused row-sums -> staging
            expo = expo_p.tile([128, 512], f32)
            for g in range(2):
                nc.scalar.activation(
                    out=expo[:, g * 256:(g + 1) * 256],
                    in_=sim[:, g * 256:(g + 1) * 256],
                    func=AF.Exp,
                    bias=negmax_sb[:, pcol + g:pcol + g + 1],
                    scale=1.0,
                    accum_out=sumexp_sb[:, pcol + g:pcol + g + 1],
                )
            # tilted row reciprocal
            rec = small_p.tile([128, 2], f32, tag="rec")
            nc.vector.reciprocal(out=rec, in_=sumexp_sb[:, pcol:pcol + 2])
            recw = small_p.tile([128, 2], f32, tag="recw")
            nc.vector.tensor_mul(out=recw, in0=rec, in1=wtilt)
            # P' = expo * recip'   (softmax probabilities, tilted; DVE
            # tensor_scalar fp32 SBUF runs in 2x mode)
            pp = p_p.tile([128, 512], f32)
            for g in range(2):
                nc.vector.tensor_scalar_mul(
                    out=pp[:, g * 256:(g + 1) * 256],
                    in0=expo[:, g * 256:(g + 1) * 256],
                    scalar1=recw[:, g:g + 1],
                )
            # PT' = transpose(P') via matmul against identity
            pt = at_ps.tile([128, 512], f32)
            for h in range(2):
                for g in range(2):
                    nc.tensor.matmul(
                        pt[:, h * 256 + g * 128: h * 256 + (g + 1) * 128],
                        lhsT=pp[:, g * 256 + h * 128: g * 256 + h * 128 + 128],
                        rhs=ident,
                        start=True, stop=True,
                    )
            cmx = small_p.tile([128, 2], f32, tag="cmx")
            nc.vector.reduce_max(
                out=cmx,
                in_=pt.rearrange("p (g c) -> p g c", g=2),
                axis=AX,
            )
            # sim diagonal -> staging (exact: mult by {0,1} mask, sum zeros)
            junk = junk_p.tile([128, 512], f32, tag="junk")
            nc.vector.tensor_tensor(out=junk, in0=sim[:, :], in1=diagmask, op=OP.mult)
            nc.vector.reduce_sum(
                out=simdiag_sb[:, pcol:pcol + 2],
                in_=junk.rearrange("p (g c) -> p g c", g=2),
                axis=AX,
            )
            # d' = exp(simdiag - rowmax) * recip'  (bit-identical to P'[c,c])
            ed = small_p.tile([128, 2], f32, tag="ed")
            for g in range(2):
                nc.scalar.activation(
                    out=ed[:, g:g + 1],
                    in_=simdiag_sb[:, pcol + g:pcol + g + 1],
                    func=AF.Exp,
                    bias=negmax_sb[:, pcol + g:pcol + g + 1],
                    scale=1.0,
                )
            # same DVE tensor_scalar path as P' so the product rounds
            # identically and the is_ge compare stays bit-exact
            dp = small_p.tile([128, 2], f32, tag="dp")
            for g in range(2):
                nc.vector.tensor_scalar_mul(
                    out=dp[:, g:g + 1], in0=ed[:, g:g + 1],
                    scalar1=recw[:, g:g + 1])
            nc.vector.tensor_tensor(
                out=corr_sb[:, pcol:pcol + 2], in0=dp, in1=cmx, op=OP.is_ge)

        # ---- main loop --------------------------------------------------
        encT = {}
        for t in range(K):
            encT[t] = load_T(enc_d[t], encT_p)
        for s in range(spc):
            ctxT = load_T(ctx_d[s], ctxT_p)
            tmax = s + K - 1
            if tmax not in encT:
                encT[tmax] = load_T(enc_d[tmax], encT_p)
            for k in range(K):
                pair(s, k, ctxT, encT[s + k])
            if s - 1 in encT:
                del encT[s - 1]

        nc.sync.dma_start(out=nmax_d[:, :], in_=negmax_sb)
        nc.sync.dma_start(out=sexp_d[:, :], in_=sumexp_sb)
        nc.sync.dma_start(out=diag_d[:, :], in_=simdiag_sb)
        nc.sync.dma_start(out=corr_d[:, :], in_=corr_sb)

    nc.compile()
    return nc


def _get_nc(spc=SPC):
    if spc not in _CACHE:
        _CACHE[spc] = _build_nc(spc)
    return _CACHE[spc]


LAST_RESULTS = None  # test harness can inspect exec_time_ns / profile


def _install_ntff_hook_shim():
    """Register the NTFF profiling hook (antenv.axon_hooks shim) so
    run_bass_kernel_spmd(trace=True) can capture a profile under axon.
    Dev-only; the graded path never calls this."""
    import sys
    import types
    import ctypes
    import contextlib

    if "antenv.axon_hooks" in sys.modules:
        return
    so_path = "/opt/axon/libaxon_pjrt.so"
    try:
        lib = ctypes.CDLL(so_path)
    except OSError:
        return
    if not hasattr(lib, "axon_start_nrt_profile"):
        return
    lib.axon_start_nrt_profile.argtypes = [ctypes.POINTER(ctypes.c_int64), ctypes.c_size_t]
    lib.axon_start_nrt_profile.restype = ctypes.c_int64
    lib.axon_stop_nrt_profile.argtypes = [ctypes.c_char_p]
    lib.axon_stop_nrt_profile.restype = ctypes.c_int64

    @contextlib.contextmanager
    def _hook(output_dir, device_ids):
        import jax
        jax.devices()
        if device_ids:
            ids = (ctypes.c_int64 * len(device_ids))(*device_ids)
            rc = lib.axon_start_nrt_profile(ids, len(device_ids))
        else:
            rc = lib.axon_start_nrt_profile(None, 0)
        if rc != 0:
            raise RuntimeError(f"axon_start_nrt_profile rc={rc}")
        try:
            yield
        finally:
            n = lib.axon_stop_nrt_profile(str(output_dir).encode())
            print(f"ntff profile: {n} file(s) written to {output_dir}")

    holder = [_hook]
    mod = types.ModuleType("antenv.axon_hooks")
    mod.get_axon_ntff_profile_hook = lambda: holder[0]
    mod.set_axon_ntff_profile_hook = lambda h: holder.__setitem__(0, h)
    sys.modules["antenv.axon_hooks"] = mod


def kernel(**inputs):
    global LAST_RESULTS
    enc = np.ascontiguousarray(np.asarray(inputs["encoded_x"], dtype=F32))
    ctxf = np.ascontiguousarray(np.asarray(inputs["context"], dtype=F32))
    wk = np.ascontiguousarray(np.asarray(inputs["Wk"], dtype=F32))
    t_in = int(inputs["timesteps_in"])
    k_out = int(inputs["timesteps_out"])
    t_ign = int(inputs["timesteps_ignore"])
    assert enc.shape == (S, B, L) and ctxf.shape == (S, B, L)
    assert wk.shape == (K, L, L)
    assert (t_in, k_out, t_ign) == (T_IN, K, 0), "kernel hardcodes these"

    from concourse.bass_utils import run_bass_kernel_spmd

    trace = bool(int(os.environ.get("CPC_TRACE", "0")))
    if trace:
        _install_ntff_hook_shim()

    nc = _get_nc()

    in_maps = []
    for i in range(NCORES):
        s0 = SPC * i
        # core 7's slices stay in range: T0 + 91 + 20 == 128
        in_maps.append({
            "enc": enc[T0 + s0: T0 + s0 + NT],
            "ctx": ctxf[T_IN + s0: T_IN + s0 + SPC],
            "wk": wk,
        })

    res = run_bass_kernel_spmd(nc, in_maps, list(range(NCORES)), trace=trace)
    LAST_RESULTS = res

    denom = B * K * STEPS
    diag_total = 0.0
    lse_total = 0.0
    corr_total = 0.0
    for i in range(NCORES):
        nvalid = 2 * K * min(SPC, STEPS - SPC * i)
        r = res.results[i]
        diag_total += r["simdiag"][:, :nvalid].astype(np.float64).sum()
        nmax = r["negmax"][:, :nvalid].astype(np.float64)
        sexp = r["sumexp"][:, :nvalid].astype(np.float64)
        lse_total += (-nmax + np.log(sexp)).sum()
        corr_total += r["corr"][:, :nvalid].astype(np.float64).sum()

    loss = np.float32(-(diag_total - lse_total) / denom)
    accuracy = np.float32(corr_total / denom)
    return (accuracy, loss)
